# revision 2
# baseline (speedup 1.0000x reference)
"""Trainium2 Bass kernel for the OPU (optical matmul + ADC quantize) module.

v2 -> v3 changes (cost-model driven):
  - The per-level LUT pass is now: fused tensor_scalar mask*value (4x DVE
    mode: out = (v == lvl) * lut_col) + tensor_tensor add (2x mode), replacing
    the 1x scalar_tensor_tensor accumulate. Level 0 writes the accumulator
    directly (no memset).
  - The whole x-side chain (fp16 convert, level loop) plus large memsets run
    on the GPSIMD (Pool) engine, concurrent with the DVE w-side chain.
Everything else (direct fp16 value map, MAGIC-seeded PSUM quantize-accumulate,
K=128 zero-padded variant matmuls, token-sharded SPMD) is as in v2.
"""
import numpy as np
from contextlib import ExitStack

import concourse.bass as bass
import concourse.bacc as bacc
import concourse.tile as tile
import concourse.mybir as mybir
from concourse import bass_utils

F32 = mybir.dt.float32
FP16 = mybir.dt.float16
BF16 = mybir.dt.bfloat16

B, S, KDIM, N = 2, 1024, 1024, 1024
BS = B * S
NCORES = 8
TOK = BS // NCORES          # 256 tokens/core
NKC = KDIM // 128           # 8 k-chunks of 128 partitions
GK = 2                      # k-chunks per build group
NG = NKC // GK              # 4 groups
WG = 1024 * GK              # w_c group width
XG = TOK * GK               # x_c group width
EQ = mybir.AluOpType.is_equal
MUL = mybir.AluOpType.mult
ADD = mybir.AluOpType.add
MAGIC = float(3 * 2**26)    # 1.5*2^27

_cache = {}


def _build():
    nc = bacc.Bacc("TRN2", target_bir_lowering=False, debug=False,
                   enable_asserts=False, num_devices=NCORES)
    xt_d = nc.dram_tensor("xt", [KDIM, TOK], F32, kind="ExternalInput").ap()
    w_d = nc.dram_tensor("w", [KDIM, N], F32, kind="ExternalInput").ap()
    g_d = nc.dram_tensor("gt", [128, 16], F32, kind="ExternalInput").ap()
    h_d = nc.dram_tensor("ht", [128, 16], F32, kind="ExternalInput").ap()
    out_d = nc.dram_tensor("out", [TOK, N], F32, kind="ExternalOutput").ap()

    with tile.TileContext(nc) as tc, ExitStack() as ctx:
        const = ctx.enter_context(tc.tile_pool(name="const", bufs=1))
        xvarp = ctx.enter_context(tc.tile_pool(name="xvar", bufs=1))
        raw = ctx.enter_context(tc.tile_pool(name="raw", bufs=2))
        ops = ctx.enter_context(tc.tile_pool(name="ops", bufs=2))
        msk = ctx.enter_context(tc.tile_pool(name="msk", bufs=2))
        outp = ctx.enter_context(tc.tile_pool(name="outp", bufs=1))
        psum = ctx.enter_context(tc.tile_pool(name="psum", bufs=1, space="PSUM"))

        g_f = const.tile([128, 16], F32, tag="g_f")
        h_f = const.tile([128, 16], F32, tag="h_f")
        nc.sync.dma_start(g_f[:], g_d[:, :])
        nc.sync.dma_start(h_f[:], h_d[:, :])
        ones = const.tile([1, 128], BF16, tag="ones")
        nc.vector.memset(ones[:], 1.0)
        mrow = const.tile([1, N], BF16, tag="mrow")
        nc.vector.memset(mrow[:], MAGIC)

        # PSUM accumulation regions, seeded with MAGIC via K=1 matmuls
        regions = {}
        for mc in range(2):
            for nh in range(2):
                acc = psum.tile([128, 512], F32, tag=f"acc{mc}{nh}")
                nc.tensor.matmul(acc[:], ones[0:1, :],
                                 mrow[0:1, 512 * nh:512 * (nh + 1)],
                                 start=True, stop=False)
                regions[(mc, nh)] = acc

        # x variants: zero-filled once (gpsimd), blocks DMA'd in per group
        xvars = []
        for rb in range(8):
            v = xvarp.tile([128, 2048], FP16, tag=f"xv{rb}")
            nc.gpsimd.memset(v[:], 0.0)
            xvars.append(v)

        for g in range(NG):
            kc0 = GK * g
            wraw = raw.tile([128, WG], F32, tag="wraw")
            xraw = raw.tile([128, XG], F32, tag="xraw")
            for i in range(GK):
                kc = kc0 + i
                nc.sync.dma_start(wraw[:, 1024 * i:1024 * (i + 1)],
                                  w_d[128 * kc:128 * (kc + 1), :])
                nc.sync.dma_start(xraw[:, TOK * i:TOK * (i + 1)],
                                  xt_d[128 * kc:128 * (kc + 1), :])
            # fp16 int parts: w on DVE, x on gpsimd
            wh = ops.tile([128, WG], FP16, tag="wh")
            xh = ops.tile([128, XG], FP16, tag="xh")
            nc.vector.tensor_copy(wh[:], wraw[:])
            nc.gpsimd.tensor_copy(xh[:], xraw[:])
            wc = ops.tile([128, WG], FP16, tag="wc")
            xc = ops.tile([128, XG], FP16, tag="xc")
            for lvl in range(16):
                c = float(lvl - 8)
                mw = msk.tile([128, WG], FP16, tag="mw")
                if lvl == 0:
                    nc.vector.tensor_scalar(wc[:], wh[:], c, h_f[:, 0:1],
                                            op0=EQ, op1=MUL)
                    nc.gpsimd.tensor_scalar(xc[:], xh[:], c, g_f[:, 0:1],
                                            op0=EQ, op1=MUL)
                    continue
                nc.vector.tensor_scalar(mw[:], wh[:], c, h_f[:, lvl:lvl + 1],
                                        op0=EQ, op1=MUL)
                nc.vector.tensor_add(wc[:], wc[:], mw[:])
                mx = msk.tile([128, XG], FP16, tag="mx")
                nc.gpsimd.tensor_scalar(mx[:], xh[:], c, g_f[:, lvl:lvl + 1],
                                        op0=EQ, op1=MUL)
                nc.gpsimd.tensor_add(xc[:], xc[:], mx[:])
            # scatter x_c rows into zero-padded variants
            for rb in range(8):
                nc.sync.dma_start(
                    xvars[rb][16 * rb:16 * (rb + 1), XG * g:XG * (g + 1)],
                    xc[16 * rb:16 * (rb + 1), :])
            # accumulate+round each r-block product into its PSUM region
            for i in range(GK):
                kc = kc0 + i
                last_kc = kc == NKC - 1
                for rb in range(8):
                    for mc in range(2):
                        for nh in range(2):
                            nc.tensor.matmul(
                                regions[(mc, nh)][:],
                                xvars[rb][:, TOK * kc + 128 * mc:
                                          TOK * kc + 128 * (mc + 1)],
                                wc[:, 1024 * i + 512 * nh:
                                   1024 * i + 512 * (nh + 1)],
                                start=False,
                                stop=(last_kc and rb == 7),
                            )

        for mc in range(2):
            o = outp.tile([128, N], F32, tag=f"o{mc}")
            for nh in range(2):
                nc.vector.tensor_scalar(
                    o[:, 512 * nh:512 * (nh + 1)], regions[(mc, nh)][:],
                    -MAGIC, None, op0=ADD)
            nc.sync.dma_start(out_d[128 * mc:128 * (mc + 1), :], o[:])

    nc.compile()
    return nc


def _prep(input, weight, vmap_lut, wmap_lut):
    xt = np.ascontiguousarray(
        input.reshape(BS, KDIM).astype(np.float32).T)      # [K, BS]
    w = np.ascontiguousarray(weight.astype(np.float32))
    lvl = np.arange(16, dtype=np.float32) - 8.0
    rows = np.arange(128) % 16
    g = np.ascontiguousarray(lvl[None, :] + vmap_lut.astype(np.float32)[rows])
    h = np.ascontiguousarray(lvl[None, :] + wmap_lut.astype(np.float32)[rows])
    return xt, w, g, h


def kernel(input, weight, vmap_lut, wmap_lut):
    if "nc" not in _cache:
        _cache["nc"] = _build()
    nc = _cache["nc"]
    xt, w, g, h = _prep(input, weight, vmap_lut, wmap_lut)
    in_maps = [
        {"xt": np.ascontiguousarray(xt[:, TOK * c:TOK * (c + 1)]),
         "w": w, "gt": g, "ht": h}
        for c in range(NCORES)
    ]
    res = bass_utils.run_bass_kernel_spmd(nc, in_maps, core_ids=list(range(NCORES)))
    out = np.concatenate([res.results[c]["out"] for c in range(NCORES)], axis=0)
    return out.reshape(B, S, N)
